# revision 32
# baseline (speedup 1.0000x reference)
"""LinearSpline activation kernel for Trainium2 (8 NeuronCores, SPMD).

Math: per channel c the reference computes a 51-knot uniform linear spline
with linear extrapolation:
  out = (c0[f] + frac*(c0[f+1]-c0[f])) / s,  f = floor(clamp(t,-25,24)),
  frac = t - f (unclamped), t = x*s/G.
As a function of t this is piecewise linear with breakpoints at the integers
-24..24 and 50 segment slopes D_{-25}..D_{24}.  We evaluate it in the ReLU
basis  g(t) = A + D_{-25}*t + sum_{j=-24}^{24} rho_j*relu(t-j),
rho_j = D_j - D_{j-1}  (all constants folded per channel on the host, incl.
the 1/s output factor).

Engine split per tile (the knot sum is ALU-bound; DVE alone floors at ~25
cyc/elem, so the high knots are farmed out to ACT+Pool+DMA):
  ACT : t = a*x, plus relu(t-j) streams for the 10 high knots j=15..24
  DVE : custom ops RINIT (A + D_lo*t + rho*relu(t+24)) and 19x RPAIR
        (acc + rho0*relu(t-j) + rho1*relu(t-j-1)) -- knots -24..14, bounds
        inline as immediates/latches so no shift streams are needed
  Pool: weights each relu stream (TS rho_j*relu_j -> tmp) and issues CCE
        accumulate-DMAs (aM += tmp) on the otherwise idle DMA engines;
        the merge into the DVE accumulator is a final accumulate-DMA
Layout: data-parallel over batch (4 per core), partition p = n2*64 + channel,
so per-channel constants ride [P,1] scalar operands.
"""

import numpy as np

# ---------------- problem constants (hardcoded; kernel.py is standalone) ---
N_BATCH, N_CH, H, W = 32, 64, 128, 128
NCORES = 8
SIZE = 51
GRID = np.float64(2.0 * 4.0 / (SIZE - 1))  # 0.16
F = 2048                    # free-dim chunk per unit
GROUPS = 2                  # batch pairs per core
FREE = H * W                # 16384 free elems per (group, partition)
UNITS = GROUPS * (FREE // F)    # 16
RING = 4                    # relu-tile ring depth

# knot split: DVE covers base + knots -24..14, Pool covers 15..24.
# Pool has no scalar_tensor_tensor on TRN2, so the weighted accumulation of
# the high knots rides the (otherwise idle) DMA engines: Pool does only the
# TS weighting ops and issues CCE accumulate-DMAs (aM += tmp); same-queue
# SWDGE ordering serializes them (verified empirically: unsynced chained
# accums produce exact results).
PAIR_J = [-23 + 2 * k for k in range(19)]   # RPAIR base knots (j, j+1)
POOL_J = list(range(15, 25))                # 10 relu knots on Pool
N_DVE = 1 + len(PAIR_J)     # 20 DVE ops per unit
N_POOL = len(POOL_J)        # Pool TS ops per unit (s_pool increments)
N_RELU = len(POOL_J)        # ACT relu streams per unit
N_ACC = len(POOL_J)         # accum-DMAs per unit: 9 knots + 1 merge
TMPR = 4                    # tmp-tile ring depth

# prm columns: 0 a_s, 1 A, 2 W_base, 3 rho_{-24}, then per-pair rho pairs,
# then Pool rho weights, then Pool relu biases (-j)
P_AS, P_A, P_WB, P_R24 = 0, 1, 2, 3
P_PAIR0 = 4
P_POOL0 = P_PAIR0 + 2 * len(PAIR_J)         # 40
P_BIAS0 = P_POOL0 + len(POOL_J)             # 52
P_COLS = P_BIAS0 + len(POOL_J)              # 64

_f32, _f64 = np.float32, np.float64
_built = {}


def _host_params(coeff, scal):
    """coeff [3264], scal [64] -> per-channel table [64, P_COLS] f32."""
    C = coeff.reshape(N_CH, SIZE).astype(_f64)
    s = scal.astype(_f64)
    D = C[:, 1:] - C[:, :-1]            # D[:, k] = slope of segment (k-25, k-24)
    rho = np.zeros((N_CH, 49), _f64)    # rho[:, j+24] for j = -24..24
    for j in range(-24, 25):
        rho[:, j + 24] = D[:, j + 25] - D[:, j + 24]
    # A: calibrate at t=0:  g(0) = c0[0] = C[:,25]
    j_neg = np.arange(-24, 0, dtype=_f64)
    A = C[:, 25] - (rho[:, :24] * (-j_neg)[None, :]).sum(axis=1)
    prm = np.zeros((N_CH, P_COLS), _f64)
    prm[:, P_AS] = s / GRID
    prm[:, P_A] = A / s
    prm[:, P_WB] = D[:, 0] / s
    prm[:, P_R24] = rho[:, 0] / s
    for k, j in enumerate(PAIR_J):
        prm[:, P_PAIR0 + 2 * k] = rho[:, j + 24] / s
        prm[:, P_PAIR0 + 2 * k + 1] = rho[:, j + 25] / s
    for i, j in enumerate(POOL_J):
        prm[:, P_POOL0 + i] = rho[:, j + 24] / s
        prm[:, P_BIAS0 + i] = -_f64(j)
    return prm.astype(_f32)


def host_eval(x, coeff, scal):
    """Numpy model of the device computation (for pre-flight validation)."""
    prm = _host_params(coeff, scal).astype(_f64)
    xs = x.astype(_f64)  # [N, C, H, W]
    a = prm[:, P_AS][None, :, None, None]
    t = xs * a
    out = (prm[:, P_A][None, :, None, None]
           + prm[:, P_WB][None, :, None, None] * t
           + prm[:, P_R24][None, :, None, None] * np.maximum(t + 24.0, 0.0))
    for k, j in enumerate(PAIR_J):
        out = out + prm[:, P_PAIR0 + 2 * k][None, :, None, None] * \
            np.maximum(t - j, 0.0)
        out = out + prm[:, P_PAIR0 + 2 * k + 1][None, :, None, None] * \
            np.maximum(t - (j + 1), 0.0)
    for i, j in enumerate(POOL_J):
        out = out + prm[:, P_POOL0 + i][None, :, None, None] * \
            np.maximum(t - j, 0.0)
    return out.astype(_f32)


def _register_ops():
    import concourse.dve_ops as dve_ops
    from concourse.dve_spec import (
        Spec, Src0, Src1, C0, C1, C2, C3, Zero, One, lower, maxx,
        _spill_c3_to_src1,
    )
    from concourse.dve_uop import DveOpSpec

    def reg(name, spec):
        for op in dve_ops.OPS:
            if op.name == name:
                return op
        row = max(dve_ops._SUB_OPCODE_FOR_NAME.values()) + 1
        assert row < 0x20
        dve_ops._SUB_OPCODE_FOR_NAME[name] = row
        uops = lower(spec, ver="v3")
        sha = DveOpSpec(name=name, opcode=row, uops=uops, rd1_en=True).sha("v3")
        op = dve_ops.DveOp(name, spec, subdim=False, uops_sha={"v3": sha})
        dve_ops.OPS.append(op)
        dve_ops.CUSTOM_DVE_SPECS[name] = spec
        return op

    # acc' = acc + C0*relu(t - j) + C1*relu(t - j - 1), imm2 = j
    rpair = reg("LS_RPAIR", Spec(body=(
        Src1
        + C0 * maxx(Src0 - C2, Zero)
        + C1 * maxx(Src0 - (C2 + One), Zero))))
    # acc0 = A + C0*t + C1*relu(t - j); A rides in1 via the C3 spill
    rinit = reg("LS_RINIT", Spec(body=_spill_c3_to_src1(
        C3 + C0 * Src0 + C1 * maxx(Src0 - C2, Zero))))
    return rpair, rinit


def _build():
    if "nc" in _built:
        return _built["nc"]
    import concourse.bass as bass
    import concourse.mybir as mybir
    from concourse.library_overlay import lower_extended_insts

    RPAIR, RINIT = _register_ops()
    F32 = mybir.dt.float32
    Ident = mybir.ActivationFunctionType.Identity
    Relu = mybir.ActivationFunctionType.Relu
    Alu = mybir.AluOpType

    nc = bass.Bass()
    x_in = nc.declare_dram_parameter("x", [GROUPS, 128, FREE], F32, isOutput=False)
    prm = nc.declare_dram_parameter("prm", [128, P_COLS], F32, isOutput=False)
    y_out = nc.declare_dram_parameter("y", [GROUPS, 128, FREE], F32, isOutput=True)

    xb = [nc.alloc_sbuf_tensor(f"xb{i}", [128, F], F32).ap() for i in range(2)]
    tb = [nc.alloc_sbuf_tensor(f"tb{i}", [128, F], F32).ap() for i in range(2)]
    rl = [nc.alloc_sbuf_tensor(f"rl{i}", [128, F], F32).ap() for i in range(RING)]
    aD = [[nc.alloc_sbuf_tensor(f"aD{s}{i}", [128, F], F32).ap() for i in range(2)]
          for s in range(2)]
    aM = [nc.alloc_sbuf_tensor(f"aM{i}", [128, F], F32).ap() for i in range(2)]
    tmp = [nc.alloc_sbuf_tensor(f"tmp{i}", [128, F], F32).ap()
           for i in range(TMPR)]
    pb = nc.alloc_sbuf_tensor("pb", [128, P_COLS], F32).ap()

    def col(i):
        return pb[:, i:i + 1]

    def unit_slice(u):
        g, ci = divmod(u, FREE // F)
        return g, ci * F

    with (nc.Block() as block,
          nc.semaphore("s_in") as s_in,
          nc.semaphore("s_t") as s_t,
          nc.semaphore("s_relu") as s_relu,
          nc.semaphore("s_dve") as s_dve,
          nc.semaphore("s_pool") as s_pool,
          nc.semaphore("s_acc") as s_acc,
          nc.semaphore("s_out") as s_out):

        @block.sync
        def _(sync):
            sync.dma_start(out=pb[:], in_=prm[:]).then_inc(s_in, 16)

            def dma_in(u):
                if u >= 2:
                    # ACT finished all reads of xb[u-2]
                    sync.wait_ge(s_relu, N_RELU * (u - 1))
                    sync.wait_ge(s_t, u - 1)
                g, off = unit_slice(u)
                sync.dma_start(out=xb[u % 2][:],
                               in_=x_in[g, :, off:off + F]).then_inc(s_in, 16)

            dma_in(0)
            dma_in(1)
            for u in range(UNITS):
                sync.wait_ge(s_acc, 16 * N_ACC * (u + 1))   # merge accum done
                g, off = unit_slice(u)
                sync.dma_start(out=y_out[g, :, off:off + F],
                               in_=aD[u % 2][len(PAIR_J) % 2][:]
                               ).then_inc(s_out, 16)
                if u + 2 < UNITS:
                    dma_in(u + 2)

        @block.scalar
        def _(scalar):
            for u in range(UNITS):
                scalar.wait_ge(s_in, 16 + 16 * (u + 1))
                if u >= 2:
                    scalar.wait_ge(s_dve, N_DVE * (u - 1))  # tb[u-2] consumed
                scalar.activation(out=tb[u % 2][:], in_=xb[u % 2][:],
                                  func=Ident, scale=col(P_AS)).then_inc(s_t, 1)
                for i, j in enumerate(POOL_J):
                    gidx = N_RELU * u + i
                    if gidx >= RING:
                        u2, i2 = divmod(gidx - RING, N_RELU)
                        scalar.wait_ge(s_pool, N_POOL * u2 + i2 + 1)
                    scalar.activation(out=rl[gidx % RING][:], in_=xb[u % 2][:],
                                      func=Relu, scale=col(P_AS),
                                      bias=col(P_BIAS0 + i)).then_inc(s_relu, 1)

        @block.vector
        def _(vector):
            for u in range(UNITS):
                vector.wait_ge(s_t, u + 1)
                if u >= 2:
                    vector.wait_ge(s_out, 16 * (u - 1))  # aD[u%2] drained
                b = aD[u % 2]
                vector._custom_dve(RINIT, out=b[0][:], in0=tb[u % 2][:],
                                   in1=col(P_A), s0=col(P_WB), s1=col(P_R24),
                                   imm2=-24.0).then_inc(s_dve, 1)
                for k, j in enumerate(PAIR_J):
                    src, dst = b[k % 2], b[(k + 1) % 2]
                    vector._custom_dve(RPAIR, out=dst[:], in0=tb[u % 2][:],
                                       in1=src[:],
                                       s0=col(P_PAIR0 + 2 * k),
                                       s1=col(P_PAIR0 + 2 * k + 1),
                                       imm2=float(j)).then_inc(s_dve, 1)
                # chain ends in b[len(PAIR_J) % 2]

        @block.gpsimd
        def _(gp):
            # accum-DMA numbering: unit u has knot accums 12u+0..12u+10 and
            # the merge accum 12u+11; tmp-writes are numbered 11u+(i-1).
            def acc_no_of_tmp(ti):
                return N_ACC * (ti // (N_RELU - 1)) + ti % (N_RELU - 1)

            for u in range(UNITS):
                m_ap = aM[u % 2][:]
                if u >= 2:
                    # aM[u%2] free once unit u-2's merge accum completed
                    gp.wait_ge(s_acc, 16 * N_ACC * (u - 1))
                for i, j in enumerate(POOL_J):
                    gp.wait_ge(s_relu, N_RELU * u + i + 1)
                    r_ap = rl[(N_RELU * u + i) % RING][:]
                    if i == 0:
                        gp.tensor_scalar(m_ap, r_ap, col(P_POOL0 + i), 0.0,
                                         Alu.mult, Alu.add).then_inc(s_pool, 1)
                    else:
                        ti = (N_RELU - 1) * u + (i - 1)
                        if ti >= TMPR:
                            gp.wait_ge(s_acc,
                                       16 * (acc_no_of_tmp(ti - TMPR) + 1))
                        t_ap = tmp[ti % TMPR][:]
                        gp.tensor_scalar(t_ap, r_ap, col(P_POOL0 + i), 0.0,
                                         Alu.mult, Alu.add).then_inc(s_pool, 1)
                        gp.dma_start(out=m_ap, in_=t_ap,
                                     accum_op=Alu.add).then_inc(s_acc, 16)
                # merge: aD[final] += aM, ordered after the knot accums by
                # same-queue SWDGE serialization
                gp.wait_ge(s_dve, N_DVE * (u + 1))
                if u >= 2:
                    gp.wait_ge(s_out, 16 * (u - 1))
                gp.dma_start(out=aD[u % 2][len(PAIR_J) % 2][:], in_=m_ap,
                             accum_op=Alu.add).then_inc(s_acc, 16)

    lower_extended_insts(nc)
    _built["nc"] = nc
    return nc


def kernel(x, coefficients_vect, scaling_coeffs_vect):
    from concourse.bass_utils import run_bass_kernel_spmd
    from concourse import bass2jax
    bass2jax.install_neuronx_cc_hook()

    x = np.ascontiguousarray(np.asarray(x, _f32))
    coeff = np.asarray(coefficients_vect, _f32).reshape(-1)
    scal = np.asarray(scaling_coeffs_vect, _f32).reshape(-1)

    prm_ch = _host_params(coeff, scal)                 # [64, P_COLS]
    prm_full = np.ascontiguousarray(np.tile(prm_ch, (2, 1)))  # [128, P_COLS]

    nb = N_BATCH // NCORES                             # 4 batches per core
    in_maps = []
    for i in range(NCORES):
        xi = x[nb * i:nb * (i + 1)].reshape(GROUPS, 128, FREE)
        in_maps.append({"x": np.ascontiguousarray(xi), "prm": prm_full})

    nc = _build()
    res = run_bass_kernel_spmd(nc, in_maps, list(range(NCORES)))

    out = np.empty((N_BATCH, N_CH, H, W), _f32)
    for i in range(NCORES):
        out[nb * i:nb * (i + 1)] = np.asarray(res.results[i]["y"]).reshape(
            nb, N_CH, H, W)
    return out
